# revision 11
# baseline (speedup 1.0000x reference)
"""GCNConv (PyG-faithful, normalize=True, add_self_loops=True) on 8 Trainium2
NeuronCores via Bass/Tile.

Strategy (1D graph/data parallel):
  - Nodes are partitioned across the 8 cores (12500 rows each, padded to
    12544 = 98 blocks of 128).
  - Phase A: each core computes h_k = x_k @ W (fp32 matmuls), scales rows by
    dinv (symmetric GCN normalization, computed host-side from the edge
    index), casts to bf16 and AllGathers the scaled table
    g = dinv[:,None] * (x @ W) into every core's DRAM.
  - Phase B: each core owns 1/8 of the destination nodes. Edges (including
    self-loops) are host-sorted by destination block; per 128-edge tile a
    dma_gather fetches g[src] rows (bf16, 256B each), a one-hot selection
    matrix sel[e,d] = (iota == dst_local[e]) (exact in bf16) feeds a TensorE
    matmul that segment-sums messages into a per-block PSUM accumulator, and
    the epilogue computes out = dinv_dst * acc + bias.

  The per-(block, chunk) tile counts are computed from the actual edge data
  at call time (the program is compiled per call), with counts maxed across
  cores so all 8 cores run an identical (SPMD) program.
"""

import sys

if "/opt/trn_rl_repo" not in sys.path:
    sys.path.insert(0, "/opt/trn_rl_repo")

import numpy as np

P = 128          # partitions / tile edge count / feature dim
NCORES = 8
WBLK = 7         # blocks per window
CHUNKS = 4       # src chunks for int16 gather indices

_PAD_DL = 300.0  # sentinel dst_local for pad edges -> all-zero sel column


def _pack(x, edge_index, weight, b):
    bias = b
    """Host-side preprocessing: sharding, normalization metadata, gather
    index packing. All numpy, vectorized."""
    x = np.ascontiguousarray(np.asarray(x, dtype=np.float32))
    ei = np.asarray(edge_index)
    weight = np.ascontiguousarray(np.asarray(weight, dtype=np.float32))
    bias = np.asarray(bias, dtype=np.float32).reshape(-1)

    n, nin = x.shape
    nout = weight.shape[1]
    assert nin == P and nout == P, (nin, nout)
    assert n % NCORES == 0, n
    nb = n // NCORES                      # nodes per core (12500)
    blocks = (nb + P - 1) // P            # blocks per core (98)
    nbp = blocks * P                      # padded nodes per core (12544)
    npad = nbp * NCORES                   # padded table rows (100352)
    wblk = WBLK if blocks % WBLK == 0 else 1
    nwin = blocks // wblk                 # windows (14)
    chunk_rows = npad // CHUNKS           # rows per chunk (25088)
    assert chunk_rows < 32768, chunk_rows

    src = ei[0].astype(np.int64)
    dst = ei[1].astype(np.int64)

    deg = np.bincount(dst, minlength=n).astype(np.float32) + 1.0
    dinv = 1.0 / np.sqrt(deg)

    loop = np.arange(n, dtype=np.int64)
    src_a = np.concatenate([src, loop])
    dst_a = np.concatenate([dst, loop])
    m = src_a.shape[0]

    core = dst_a // nb
    dlc = dst_a - core * nb               # dst local to core
    blk = dlc >> 7
    dl = (dlc & 127).astype(np.float32)
    grow = (src_a // nb) * nbp + (src_a % nb)   # padded global row of src
    chunk = grow // chunk_rows
    rel = (grow % chunk_rows).astype(np.int16)

    key = (core * blocks + blk) * CHUNKS + chunk
    order = np.argsort(key, kind="stable")
    karr = core[order]
    relarr = rel[order]
    dlarr = dl[order]
    gkey = key[order]

    counts = np.bincount(key, minlength=NCORES * blocks * CHUNKS).reshape(
        NCORES, blocks, CHUNKS
    )
    t_bc = -(-counts.max(axis=0) // P)    # [blocks, CHUNKS] tiles per slot

    # global tile layout: for w in windows: for c in chunks: for b in window
    tile_off = np.zeros((blocks, CHUNKS), np.int64)
    sec_start = np.zeros((nwin, CHUNKS), np.int64)
    wbase = np.zeros(nwin + 1, np.int64)
    col = 0
    for w in range(nwin):
        wbase[w] = col
        for c in range(CHUNKS):
            sec_start[w, c] = col
            for b in range(w * wblk, (w + 1) * wblk):
                tile_off[b, c] = col
                col += t_bc[b, c]
    t_total = int(col)
    wbase[nwin] = col

    # scatter edges into per-core packed arrays
    gs = np.zeros(NCORES * blocks * CHUNKS, np.int64)
    gs[1:] = np.cumsum(counts.ravel())[:-1]
    rank = np.arange(m, dtype=np.int64) - gs[gkey]
    base_flat = (tile_off * P).ravel()    # same for all cores
    dest = base_flat[(gkey % (blocks * CHUNKS))] + rank

    idx_lin = np.zeros((NCORES, t_total * P), np.int16)
    dl_lin = np.full((NCORES, t_total * P), _PAD_DL, np.float32)
    idx_lin[karr, dest] = relarr
    dl_lin[karr, dest] = dlarr

    # wrap-16 + replicate to 128 partitions for dma_gather idx layout
    l16 = t_total * P // 16
    idx_w = idx_lin.reshape(NCORES, l16, 16).transpose(0, 2, 1)  # [8,16,L16]
    idx_pack = np.ascontiguousarray(np.tile(idx_w, (1, NCORES, 1)))  # [8,128,L16]
    dl_pack = np.ascontiguousarray(
        dl_lin.reshape(NCORES, t_total, P).transpose(0, 2, 1)
    )  # [8,128,T]

    # per-core xT, dinv, out padding
    xt = np.zeros((NCORES, P, nbp), np.float32)
    dinv_t = np.zeros((NCORES, P, blocks), np.float32)
    for k in range(NCORES):
        xs = x[k * nb : (k + 1) * nb]
        xt[k, :, :nb] = xs.T
        dv = np.zeros(nbp, np.float32)
        dv[:nb] = dinv[k * nb : (k + 1) * nb]
        dinv_t[k] = dv.reshape(blocks, P).T
    bias_rep = np.ascontiguousarray(np.tile(bias[None, :], (P, 1)))

    meta = dict(
        n=n, nb=nb, blocks=blocks, nbp=nbp, npad=npad, nwin=nwin, wblk=wblk,
        chunk_rows=chunk_rows, t_bc=t_bc, tile_off=tile_off,
        sec_start=sec_start, wbase=wbase, t_total=t_total, l16=l16,
    )
    in_maps = [
        {
            "xt": xt[k],
            "w_in": weight,
            "bias": bias_rep,
            "dinv": dinv_t[k],
            "idxp": idx_pack[k],
            "dlv": dl_pack[k],
        }
        for k in range(NCORES)
    ]
    return meta, in_maps


def _build_program(meta):
    from concourse import bass, bacc, mybir
    import concourse.tile as tile

    blocks = meta["blocks"]
    nbp = meta["nbp"]
    npad = meta["npad"]
    nwin = meta["nwin"]
    wblk = meta["wblk"]
    chunk_rows = meta["chunk_rows"]
    t_bc = meta["t_bc"]
    tile_off = meta["tile_off"]
    wbase = meta["wbase"]
    t_total = meta["t_total"]
    l16 = meta["l16"]
    jmax = int((wbase[1:] - wbase[:-1]).max())

    f32 = mybir.dt.float32
    bf16 = mybir.dt.bfloat16

    nc = bacc.Bacc()
    xt_in = nc.declare_dram_parameter("xt", [P, nbp], f32, isOutput=False)
    w_in = nc.declare_dram_parameter("w_in", [P, P], f32, isOutput=False)
    bias_in = nc.declare_dram_parameter("bias", [P, P], f32, isOutput=False)
    dinv_in = nc.declare_dram_parameter("dinv", [P, blocks], f32, isOutput=False)
    idx_in = nc.declare_dram_parameter("idxp", [P, l16], mybir.dt.int16, isOutput=False)
    dl_in = nc.declare_dram_parameter("dlv", [P, t_total], f32, isOutput=False)
    out_ext = nc.declare_dram_parameter("out", [nbp, P], f32, isOutput=True)

    h_shard = nc.dram_tensor("h_shard", [nbp, P], bf16)
    g_table = nc.dram_tensor("g_table", [npad, P], bf16, addr_space="Shared")

    with tile.TileContext(nc) as tc:
        with (
            tc.tile_pool(name="const", bufs=1) as cpool,
            tc.tile_pool(name="work", bufs=4) as wpool,
            tc.tile_pool(name="msgp", bufs=2) as mpool,
            tc.tile_pool(name="selp", bufs=8) as spool,
            tc.tile_pool(name="psA", bufs=2, space="PSUM") as psA,
            tc.tile_pool(name="psB", bufs=4, space="PSUM") as psB,
        ):
            # constants / metadata loads
            xt_sb = cpool.tile([P, nbp], f32, tag="xt")
            for i in range(4):
                s = nbp // 4
                nc.sync.dma_start(
                    out=xt_sb[:, i * s : (i + 1) * s],
                    in_=xt_in[:, i * s : (i + 1) * s],
                )
            w_sb = cpool.tile([P, P], f32, tag="w")
            nc.sync.dma_start(out=w_sb[:], in_=w_in[:])
            bias_sb = cpool.tile([P, P], f32, tag="bias")
            nc.sync.dma_start(out=bias_sb[:], in_=bias_in[:])
            dinv_sb = cpool.tile([P, blocks], f32, tag="dinv")
            nc.sync.dma_start(out=dinv_sb[:], in_=dinv_in[:])
            idx_sb = cpool.tile([P, l16], mybir.dt.int16, tag="idx")
            for i in range(4):
                s = l16 // 4
                e = l16 if i == 3 else (i + 1) * s
                nc.sync.dma_start(out=idx_sb[:, i * s : e], in_=idx_in[:, i * s : e])
            dl_sb = cpool.tile([P, t_total], f32, tag="dl")
            nc.sync.dma_start(out=dl_sb[:], in_=dl_in[:])
            iota_i = cpool.tile([P, P], mybir.dt.int32, tag="iotai")
            nc.gpsimd.iota(iota_i[:], pattern=[[1, P]], base=0, channel_multiplier=0)
            iota_f = cpool.tile([P, P], f32, tag="iotaf")
            nc.vector.tensor_copy(out=iota_f[:], in_=iota_i[:])

            # ---- phase A: h = x @ W, scale by dinv, cast bf16, allgather
            for t in range(blocks):
                ph = psA.tile([P, P], f32, tag="ph")
                nc.tensor.matmul(
                    out=ph[:],
                    lhsT=xt_sb[:, t * P : (t + 1) * P],
                    rhs=w_sb[:],
                    start=True,
                    stop=True,
                )
                ht = wpool.tile([P, P], bf16, tag="ht")
                nc.vector.tensor_scalar(
                    out=ht[:],
                    in0=ph[:],
                    scalar1=dinv_sb[:, t : t + 1],
                    scalar2=None,
                    op0=mybir.AluOpType.mult,
                )
                nc.sync.dma_start(out=h_shard[t * P : (t + 1) * P, :], in_=ht[:])

            nc.gpsimd.collective_compute(
                "AllGather",
                mybir.AluOpType.bypass,
                replica_groups=[list(range(NCORES))],
                ins=[h_shard[:]],
                outs=[g_table[:]],
            )

            # ---- phase B: gather + one-hot segment matmul per dst block
            for w in range(nwin):
                jw = int(wbase[w + 1] - wbase[w])
                msg = mpool.tile([P, jmax, P], bf16, tag="msg")
                for c in range(CHUNKS):
                    sec0 = None
                    seclen = 0
                    for b in range(w * wblk, (w + 1) * wblk):
                        if t_bc[b, c] > 0:
                            if sec0 is None:
                                sec0 = int(tile_off[b, c])
                            seclen += int(t_bc[b, c])
                    if seclen == 0:
                        continue
                    lo = sec0 - int(wbase[w])
                    nc.gpsimd.dma_gather(
                        out_ap=msg[:, lo : lo + seclen, :],
                        in_ap=g_table[c * chunk_rows : (c + 1) * chunk_rows, :],
                        idxs_ap=idx_sb[:, sec0 * 8 : (sec0 + seclen) * 8],
                        num_idxs=seclen * P,
                        num_idxs_reg=seclen * P,
                        elem_size=P,
                        single_packet=False,
                    )
                for b in range(w * wblk, (w + 1) * wblk):
                    ntiles = int(t_bc[b].sum())
                    assert ntiles > 0
                    acc = psB.tile([P, P], f32, tag="acc")
                    ti = 0
                    for c in range(CHUNKS):
                        for t in range(int(t_bc[b, c])):
                            gt = int(tile_off[b, c]) + t
                            mcol = gt - int(wbase[w])
                            sel = spool.tile([P, P], bf16, tag="sel")
                            nc.vector.tensor_scalar(
                                out=sel[:],
                                in0=iota_f[:],
                                scalar1=dl_sb[:, gt : gt + 1],
                                scalar2=None,
                                op0=mybir.AluOpType.is_equal,
                            )
                            nc.tensor.matmul(
                                out=acc[:],
                                lhsT=sel[:],
                                rhs=msg[:, mcol, :],
                                start=(ti == 0),
                                stop=(ti == ntiles - 1),
                            )
                            ti += 1
                    osb = wpool.tile([P, P], f32, tag="osb")
                    nc.any.tensor_scalar(
                        out=osb[:],
                        in0=acc[:],
                        scalar1=dinv_sb[:, b : b + 1],
                        scalar2=None,
                        op0=mybir.AluOpType.mult,
                    )
                    nc.vector.tensor_tensor(
                        out=osb[:],
                        in0=osb[:],
                        in1=bias_sb[:],
                        op=mybir.AluOpType.add,
                    )
                    nc.sync.dma_start(out=out_ext[b * P : (b + 1) * P, :], in_=osb[:])

    nc.finalize()
    return nc


def _run(inputs, trace=False, trace_cores=None):
    from concourse.bass_utils import run_bass_kernel_spmd

    meta, in_maps = _pack(**inputs)
    nc = _build_program(meta)
    res = run_bass_kernel_spmd(
        nc,
        in_maps,
        list(range(NCORES)),
        trace=trace,
        trace_cores=trace_cores,
    )
    n, nb, nbp = meta["n"], meta["nb"], meta["nbp"]
    out = np.empty((n, P), np.float32)
    for k in range(NCORES):
        out[k * nb : (k + 1) * nb] = np.asarray(res.results[k]["out"])[:nb]
    return out, res


def kernel(x, edge_index, weight, b):
    out, _ = _run(dict(x=x, edge_index=edge_index, weight=weight, b=b))
    return out


if __name__ == "__main__":
    # quick self-test with random data (small-ish check against numpy)
    rng = np.random.default_rng(0)
    n, e = 100000, 1600000
    x = rng.standard_normal((n, P), dtype=np.float32)
    ei = rng.integers(0, n, (2, e)).astype(np.int64)
    w = (rng.standard_normal((P, P)) / np.sqrt(P)).astype(np.float32)
    bb = (rng.standard_normal(P) * 0.02).astype(np.float32)
    out = kernel(x, ei, w, bb)
    print("out", out.shape, out.dtype)


# revision 12
# speedup vs baseline: 1.9046x; 1.9046x over previous
"""GCNConv (PyG-faithful, normalize=True, add_self_loops=True) on 8 Trainium2
NeuronCores via Bass/Tile.

Strategy (1D graph/data parallel):
  - Nodes are partitioned across the 8 cores (12500 rows each, padded to
    12544 = 98 blocks of 128).
  - Phase A: each core computes h_k = x_k @ W (fp32 matmuls), scales rows by
    dinv (symmetric GCN normalization, computed host-side from the edge
    index), casts to bf16 and AllGathers the scaled table
    g = dinv[:,None] * (x @ W) into every core's DRAM.
  - Phase B: each core owns 1/8 of the destination nodes. Edges (including
    self-loops) are host-sorted by destination block; per 128-edge tile a
    dma_gather (SWDGE, 4 queues round-robin) fetches g[src] rows (bf16), a
    host-precomputed one-hot selection tile (fp8, streamed from DRAM via
    HWDGE) feeds a TensorE matmul that segment-sums messages into a
    per-block PSUM accumulator. The epilogue scales by dinv_dst on ScalarE
    (PSUM->SBUF copy), adds bias per window on VectorE, and stores one
    window (896 nodes) per DMA.

  Per-(block, chunk) tile counts are computed from the actual edge data at
  call time (the program is compiled per call), maxed across cores so all 8
  cores run an identical (SPMD) program.
"""

import sys

if "/opt/trn_rl_repo" not in sys.path:
    sys.path.insert(0, "/opt/trn_rl_repo")

import numpy as np

P = 128          # partitions / tile edge count / feature dim
NCORES = 8
WBLK = 7         # blocks per window
CHUNKS = 4       # src chunks for int16 gather indices

_PAD_DL = 300    # sentinel dst_local for pad edges -> all-zero sel column


def _pack(x, edge_index, weight, b):
    """Host-side preprocessing: sharding, normalization metadata, gather
    index packing, one-hot sel tiles. All numpy, vectorized."""
    import ml_dtypes

    bias = b
    x = np.ascontiguousarray(np.asarray(x, dtype=np.float32))
    ei = np.asarray(edge_index)
    weight = np.ascontiguousarray(np.asarray(weight, dtype=np.float32))
    bias = np.asarray(bias, dtype=np.float32).reshape(-1)

    n, nin = x.shape
    nout = weight.shape[1]
    assert nin == P and nout == P, (nin, nout)
    assert n % NCORES == 0, n
    nb = n // NCORES                      # nodes per core (12500)
    blocks = (nb + P - 1) // P            # blocks per core (98)
    nbp = blocks * P                      # padded nodes per core (12544)
    npad = nbp * NCORES                   # padded table rows (100352)
    wblk = WBLK if blocks % WBLK == 0 else 1
    nwin = blocks // wblk                 # windows (14)
    chunk_rows = npad // CHUNKS           # rows per chunk (25088)
    assert chunk_rows < 32768, chunk_rows

    src = ei[0].astype(np.int64)
    dst = ei[1].astype(np.int64)

    deg = np.bincount(dst, minlength=n).astype(np.float32) + 1.0
    dinv = 1.0 / np.sqrt(deg)

    loop = np.arange(n, dtype=np.int64)
    src_a = np.concatenate([src, loop])
    dst_a = np.concatenate([dst, loop])
    m = src_a.shape[0]

    core = dst_a // nb
    dlc = dst_a - core * nb               # dst local to core
    blk = dlc >> 7
    dl = (dlc & 127).astype(np.int64)
    grow = (src_a // nb) * nbp + (src_a % nb)   # padded global row of src
    chunk = grow // chunk_rows
    rel = (grow % chunk_rows).astype(np.int16)

    key = (core * blocks + blk) * CHUNKS + chunk
    order = np.argsort(key, kind="stable")
    karr = core[order]
    relarr = rel[order]
    dlarr = dl[order]
    gkey = key[order]

    counts = np.bincount(key, minlength=NCORES * blocks * CHUNKS).reshape(
        NCORES, blocks, CHUNKS
    )
    t_bc = -(-counts.max(axis=0) // P)    # [blocks, CHUNKS] tiles per slot

    # global tile layout: for w in windows: for c in chunks: for b in window
    tile_off = np.zeros((blocks, CHUNKS), np.int64)
    wbase = np.zeros(nwin + 1, np.int64)
    col = 0
    for w in range(nwin):
        wbase[w] = col
        for c in range(CHUNKS):
            for bb in range(w * wblk, (w + 1) * wblk):
                tile_off[bb, c] = col
                col += t_bc[bb, c]
    t_total = int(col)
    wbase[nwin] = col

    # scatter edges into per-core packed arrays
    gs = np.zeros(NCORES * blocks * CHUNKS, np.int64)
    gs[1:] = np.cumsum(counts.ravel())[:-1]
    rank = np.arange(m, dtype=np.int64) - gs[gkey]
    base_flat = (tile_off * P).ravel()    # same for all cores
    dest = base_flat[(gkey % (blocks * CHUNKS))] + rank

    idx_lin = np.zeros((NCORES, t_total * P), np.int16)
    dl_lin = np.full((NCORES, t_total * P), _PAD_DL, np.int16)
    idx_lin[karr, dest] = relarr
    dl_lin[karr, dest] = dlarr.astype(np.int16)

    # wrap-16 + replicate to 128 partitions for dma_gather idx layout
    l16 = t_total * P // 16
    idx_w = idx_lin.reshape(NCORES, l16, 16).transpose(0, 2, 1)  # [8,16,L16]
    idx_pack = np.ascontiguousarray(np.tile(idx_w, (1, NCORES, 1)))  # [8,128,L16]

    # host-precomputed one-hot sel tiles, fp8: sel[e, gt, d] = (dl[gt,e] == d)
    sel_pack = np.empty((NCORES, P, t_total * P), ml_dtypes.float8_e4m3)
    dgrid = np.arange(P, dtype=np.int16)[None, None, :]
    for k in range(NCORES):
        dlr = dl_lin[k].reshape(t_total, P)          # [gt, e]
        sel_k = dlr.T[:, :, None] == dgrid           # [e, gt, d] bool
        sel_pack[k] = sel_k.reshape(P, t_total * P).astype(ml_dtypes.float8_e4m3)

    # per-core xT, dinv
    xt = np.zeros((NCORES, P, nbp), np.float32)
    dinv_t = np.zeros((NCORES, P, blocks), np.float32)
    for k in range(NCORES):
        xs = x[k * nb : (k + 1) * nb]
        xt[k, :, :nb] = xs.T
        dv = np.zeros(nbp, np.float32)
        dv[:nb] = dinv[k * nb : (k + 1) * nb]
        dinv_t[k] = dv.reshape(blocks, P).T
    bias_rep = np.ascontiguousarray(np.tile(bias[None, :], (P, 1)))

    meta = dict(
        n=n, nb=nb, blocks=blocks, nbp=nbp, npad=npad, nwin=nwin, wblk=wblk,
        chunk_rows=chunk_rows, t_bc=t_bc, tile_off=tile_off,
        wbase=wbase, t_total=t_total, l16=l16,
    )
    in_maps = [
        {
            "xt": xt[k],
            "w_in": weight,
            "bias": bias_rep,
            "dinv": dinv_t[k],
            "idxp": idx_pack[k],
            "selp": sel_pack[k],
        }
        for k in range(NCORES)
    ]
    return meta, in_maps


def _build_program(meta):
    from concourse import bass, bacc, mybir
    import concourse.tile as tile

    blocks = meta["blocks"]
    nbp = meta["nbp"]
    npad = meta["npad"]
    nwin = meta["nwin"]
    wblk = meta["wblk"]
    chunk_rows = meta["chunk_rows"]
    t_bc = meta["t_bc"]
    tile_off = meta["tile_off"]
    wbase = meta["wbase"]
    t_total = meta["t_total"]
    l16 = meta["l16"]
    jmax = int((wbase[1:] - wbase[:-1]).max())
    selmax = int(t_bc.sum(axis=1).max())  # max tiles per block

    f32 = mybir.dt.float32
    bf16 = mybir.dt.bfloat16
    fp8 = mybir.dt.float8e4

    nc = bacc.Bacc(num_swdge_queues=4)
    xt_in = nc.declare_dram_parameter("xt", [P, nbp], f32, isOutput=False)
    w_in = nc.declare_dram_parameter("w_in", [P, P], f32, isOutput=False)
    bias_in = nc.declare_dram_parameter("bias", [P, P], f32, isOutput=False)
    dinv_in = nc.declare_dram_parameter("dinv", [P, blocks], f32, isOutput=False)
    idx_in = nc.declare_dram_parameter("idxp", [P, l16], mybir.dt.int16, isOutput=False)
    sel_in = nc.declare_dram_parameter("selp", [P, t_total * P], fp8, isOutput=False)
    out_ext = nc.declare_dram_parameter("out", [nbp, P], f32, isOutput=True)

    h_shard = nc.dram_tensor("h_shard", [nbp, P], bf16)
    g_table = nc.dram_tensor("g_table", [npad, P], bf16, addr_space="Shared")

    with tile.TileContext(nc) as tc:
        with (
            tc.tile_pool(name="const", bufs=1) as cpool,
            tc.tile_pool(name="work", bufs=4) as wpool,
            tc.tile_pool(name="msgp", bufs=2) as mpool,
            tc.tile_pool(name="selp", bufs=3) as spool,
            tc.tile_pool(name="outp", bufs=2) as opool,
            tc.tile_pool(name="psA", bufs=2, space="PSUM") as psA,
            tc.tile_pool(name="psB", bufs=4, space="PSUM") as psB,
        ):
            # constants / metadata loads
            w_sb = cpool.tile([P, P], f32, tag="w")
            nc.sync.dma_start(out=w_sb[:], in_=w_in[:])
            bias_sb = cpool.tile([P, P], f32, tag="bias")
            nc.sync.dma_start(out=bias_sb[:], in_=bias_in[:])
            dinv_sb = cpool.tile([P, blocks], f32, tag="dinv")
            nc.sync.dma_start(out=dinv_sb[:], in_=dinv_in[:])
            idx_sb = cpool.tile([P, l16], mybir.dt.int16, tag="idx")
            for i in range(4):
                s = l16 // 4
                e = l16 if i == 3 else (i + 1) * s
                nc.sync.dma_start(out=idx_sb[:, i * s : e], in_=idx_in[:, i * s : e])

            # ---- phase A: h = x @ W, scale by dinv, cast bf16, allgather
            for t in range(blocks):
                xt_t = wpool.tile([P, P], f32, tag="xt")
                nc.sync.dma_start(out=xt_t[:], in_=xt_in[:, t * P : (t + 1) * P])
                ph = psA.tile([P, P], f32, tag="ph")
                nc.tensor.matmul(
                    out=ph[:], lhsT=xt_t[:], rhs=w_sb[:], start=True, stop=True
                )
                ht = wpool.tile([P, P], bf16, tag="ht")
                nc.vector.tensor_scalar(
                    out=ht[:],
                    in0=ph[:],
                    scalar1=dinv_sb[:, t : t + 1],
                    scalar2=None,
                    op0=mybir.AluOpType.mult,
                )
                nc.sync.dma_start(out=h_shard[t * P : (t + 1) * P, :], in_=ht[:])

            nc.gpsimd.collective_compute(
                "AllGather",
                mybir.AluOpType.bypass,
                replica_groups=[list(range(NCORES))],
                ins=[h_shard[:]],
                outs=[g_table[:]],
            )

            # ---- phase B: gather + one-hot segment matmul per dst block
            for w in range(nwin):
                msg = mpool.tile([P, jmax, P], bf16, tag="msg")
                for c in range(CHUNKS):
                    sec0 = None
                    seclen = 0
                    for bb in range(w * wblk, (w + 1) * wblk):
                        if t_bc[bb, c] > 0:
                            if sec0 is None:
                                sec0 = int(tile_off[bb, c])
                            seclen += int(t_bc[bb, c])
                    if seclen == 0:
                        continue
                    lo = sec0 - int(wbase[w])
                    nc.gpsimd.dma_gather(
                        out_ap=msg[:, lo : lo + seclen, :],
                        in_ap=g_table[c * chunk_rows : (c + 1) * chunk_rows, :],
                        idxs_ap=idx_sb[:, sec0 * 8 : (sec0 + seclen) * 8],
                        num_idxs=seclen * P,
                        num_idxs_reg=seclen * P,
                        elem_size=P,
                        single_packet=False,
                        queue_num=c,
                    )
                osb_w = opool.tile([P, wblk, P], f32, tag="osbw")
                for j, bb in enumerate(range(w * wblk, (w + 1) * wblk)):
                    ntiles = int(t_bc[bb].sum())
                    assert ntiles > 0
                    # stream this block's sel tiles from DRAM (HWDGE)
                    selb = spool.tile([P, selmax * P], fp8, tag="selb")
                    si = 0
                    for c in range(CHUNKS):
                        tb = int(t_bc[bb, c])
                        if tb == 0:
                            continue
                        g0 = int(tile_off[bb, c])
                        nc.sync.dma_start(
                            out=selb[:, si * P : (si + tb) * P],
                            in_=sel_in[:, g0 * P : (g0 + tb) * P],
                        )
                        si += tb
                    acc = psB.tile([P, P], f32, tag="acc")
                    ti = 0
                    for c in range(CHUNKS):
                        tb = int(t_bc[bb, c])
                        for t in range(tb):
                            gt = int(tile_off[bb, c]) + t
                            mcol = gt - int(wbase[w])
                            nc.tensor.matmul(
                                out=acc[:],
                                lhsT=selb[:, ti * P : (ti + 1) * P],
                                rhs=msg[:, mcol, :],
                                start=(ti == 0),
                                stop=(ti == ntiles - 1),
                            )
                            ti += 1
                    # epilogue: scale by dinv_dst on ScalarE (PSUM -> SBUF)
                    nc.scalar.activation(
                        out=osb_w[:, j, :],
                        in_=acc[:],
                        func=mybir.ActivationFunctionType.Copy,
                        scale=dinv_sb[:, bb : bb + 1],
                    )
                # bias add for the whole window on VectorE, then store
                nc.vector.tensor_tensor(
                    out=osb_w[:],
                    in0=osb_w[:],
                    in1=bias_sb[:].unsqueeze(1).to_broadcast([P, wblk, P]),
                    op=mybir.AluOpType.add,
                )
                nc.sync.dma_start(
                    out=out_ext[w * wblk * P : (w + 1) * wblk * P, :].rearrange(
                        "(j p) f -> p j f", p=P
                    ),
                    in_=osb_w[:],
                )

    nc.finalize()
    return nc


def _run(inputs, trace=False, trace_cores=None):
    from concourse.bass_utils import run_bass_kernel_spmd

    meta, in_maps = _pack(**inputs)
    nc = _build_program(meta)
    res = run_bass_kernel_spmd(
        nc,
        in_maps,
        list(range(NCORES)),
        trace=trace,
        trace_cores=trace_cores,
    )
    n, nb, nbp = meta["n"], meta["nb"], meta["nbp"]
    out = np.empty((n, P), np.float32)
    for k in range(NCORES):
        out[k * nb : (k + 1) * nb] = np.asarray(res.results[k]["out"])[:nb]
    return out, res


def kernel(x, edge_index, weight, b):
    out, _ = _run(dict(x=x, edge_index=edge_index, weight=weight, b=b))
    return out


if __name__ == "__main__":
    rng = np.random.default_rng(0)
    n, e = 100000, 1600000
    x = rng.standard_normal((n, P), dtype=np.float32)
    ei = rng.integers(0, n, (2, e)).astype(np.int64)
    w = (rng.standard_normal((P, P)) / np.sqrt(P)).astype(np.float32)
    bb = (rng.standard_normal(P) * 0.02).astype(np.float32)
    out = kernel(x, ei, w, bb)
    print("out", out.shape, out.dtype)
